# revision 19
# baseline (speedup 1.0000x reference)
"""GNN message-passing MLP on 8 Trainium2 NeuronCores.

Computes, for each of 2 "mc" embedding tables x (shape [N, 128]) and each of
500K edges (src, dst):
    y = relu(x[src] @ W1a + x[dst] @ W1b + b1) @ W2 + b2        # [2, E, 128]

Distribution: edge-parallel across 8 cores; node table + weights replicated
per core (no collectives).

Per-core kernel:
- The two mc tables are interleaved into one fp16 table [N, 256] (one 512B
  row per node serves both mc) and gathered with the GPSIMD dma_gather
  custom instruction in transpose mode, which lands gathered rows
  feature-major in SBUF ([128 feat, mc, edge]) - exactly the matmul layout.
- dma_gather indices are int16, so the table is addressed in 4 windows of
  25000 rows. The host groups each core's edges by (src window, dst window)
  into 16 groups (padded to a fixed quota) so each gather instruction hits a
  single window; outputs are produced in grouped order and inverse-permuted
  on the host.
- Layer 1 runs weight-stationary (out = W1c.T @ xT -> hT in PSUM), bias+relu
  is fused on the scalar engine (bias is per-partition in hT layout), layer 2
  contracts hT blocks against W2 chunks, and b2 is added during the
  PSUM->SBUF copy on the vector engine.
"""

import os
import sys

import numpy as np

for _p in ("/opt/trn_rl_repo", "/root/.axon_site/_ro/trn_rl_repo"):
    if os.path.isdir(_p) and _p not in sys.path:
        sys.path.insert(0, _p)

import concourse.bass as bass
import concourse.mybir as mybir
import concourse.tile as tile
from concourse import bacc
from concourse.bass_utils import run_bass_kernel_spmd
from concourse.masks import make_identity  # noqa: F401  (kept for variants)

# Problem constants (hardcoded per harness contract).
N_NODES = 100000
E_TOTAL = 500000
D = 128          # input feature dim
H = 256          # hidden dim
O = 128          # output dim
MC = 2           # number of embedding tables
CORES = 8
P = 128

# Windowed gather layout.
WN = 25000                   # nodes per index window (int16-addressable)
WC = 4                       # windows
NG = WC * WC                 # (src window, dst window) groups
QUOTA = 4352                 # padded edges per group (mean 3906, +7.4 sigma)
EC_DEV = NG * QUOTA          # padded edges per core (69632)
E_CORE = E_TOTAL // CORES    # real edges per core (62500)
CHUNKS = (2048, 2048, 256)   # gather instruction sizes per group (sum=QUOTA)
SUBW = 512                   # compute batch width (edges per L1 matmul)

_CACHE = {}
_last_in_maps = None


def _build(repeats=1):
    f16 = mybir.dt.float16
    f32 = mybir.dt.float32
    i16 = mybir.dt.int16

    idx_cols_per_group = 2 * (QUOTA // 16)       # src + dst, wrapped by 16
    idx_cols = NG * idx_cols_per_group           # 8704

    nc = bacc.Bacc("TRN2", target_bir_lowering=False, num_devices=CORES)
    tab = nc.declare_dram_parameter("tab", [N_NODES, MC * D], f16, isOutput=False)
    idx = nc.declare_dram_parameter("idx", [P, idx_cols], i16, isOutput=False)
    w1 = nc.declare_dram_parameter("w1", [2, D, H], f16, isOutput=False)
    w2 = nc.declare_dram_parameter("w2", [H // P, P, O], f16, isOutput=False)
    b1 = nc.declare_dram_parameter("b1", [H // P, P], f32, isOutput=False)
    b2 = nc.declare_dram_parameter("b2", [P, SUBW], f32, isOutput=False)
    y = nc.declare_dram_parameter("y", [MC, EC_DEV, O], f32, isOutput=True)

    relu = mybir.ActivationFunctionType.Relu
    add = mybir.AluOpType.add

    with tile.TileContext(nc) as tc:
        with (
            tc.tile_pool(name="const", bufs=1) as cpool,
            tc.tile_pool(name="xg", bufs=2) as xgpool,
            tc.tile_pool(name="ht", bufs=3) as htpool,
            tc.tile_pool(name="yo", bufs=3) as yopool,
            tc.tile_pool(name="ph", bufs=2, space="PSUM") as phpool,
            tc.tile_pool(name="py", bufs=2, space="PSUM") as pypool,
        ):
            w1_sb = cpool.tile([P, 2, H], f16)       # [d, a/b, h]
            nc.sync.dma_start(w1_sb[:], w1.rearrange("a d h -> d a h"))
            w2_sb = cpool.tile([P, H // P, O], f16)  # [h_in_chunk, chunk, o]
            nc.sync.dma_start(w2_sb[:], w2.rearrange("c h o -> h c o"))
            b1_sb = cpool.tile([P, H // P], f32)
            nc.sync.dma_start(b1_sb[:], b1.rearrange("c p -> p c"))
            b2_sb = cpool.tile([P, SUBW], f32)
            nc.sync.dma_start(b2_sb[:], b2[:])
            ix_all = cpool.tile([P, idx_cols], i16)
            nc.sync.dma_start(ix_all[:], idx[:])

            for _rep in range(repeats):
                for g in range(NG):
                    ws, wd = g // WC, g % WC
                    gbase = g * QUOTA                    # edge-stream offset
                    icol = g * idx_cols_per_group        # idx-column offset
                    ebase = gbase
                    for ni in CHUNKS:
                        s_cols = ni // 16
                        # Gather src rows (transpose mode -> feature-major).
                        xst = xgpool.tile([P, MC, ni], f16, tag="xst")
                        nc.gpsimd.dma_gather(
                            out_ap=xst[:],
                            in_ap=tab[ws * WN:(ws + 1) * WN, :],
                            idxs_ap=ix_all[:, icol:icol + s_cols],
                            num_idxs=ni,
                            num_idxs_reg=ni,
                            elem_size=MC * D,
                            transpose=True,
                            single_packet=False,
                        )
                        xdt = xgpool.tile([P, MC, ni], f16, tag="xdt")
                        nc.gpsimd.dma_gather(
                            out_ap=xdt[:],
                            in_ap=tab[wd * WN:(wd + 1) * WN, :],
                            idxs_ap=ix_all[:, icol + QUOTA // 16:
                                           icol + QUOTA // 16 + s_cols],
                            num_idxs=ni,
                            num_idxs_reg=ni,
                            elem_size=MC * D,
                            transpose=True,
                            single_packet=False,
                        )
                        icol += s_cols

                        for o_ in range(0, ni, SUBW):
                            wc = min(SUBW, ni - o_)
                            for mc in range(MC):
                                hts = []
                                for c in range(H // P):
                                    ph = phpool.tile([P, SUBW], f32,
                                                     tag=f"ph{c}")
                                    nc.tensor.matmul(
                                        ph[:, :wc],
                                        lhsT=w1_sb[:, 0, c * P:(c + 1) * P],
                                        rhs=xst[:, mc, o_:o_ + wc],
                                        start=True, stop=False,
                                    )
                                    nc.tensor.matmul(
                                        ph[:, :wc],
                                        lhsT=w1_sb[:, 1, c * P:(c + 1) * P],
                                        rhs=xdt[:, mc, o_:o_ + wc],
                                        start=False, stop=True,
                                    )
                                    ht = htpool.tile([P, SUBW], f16,
                                                     tag=f"ht{c}")
                                    nc.scalar.activation(
                                        ht[:, :wc], ph[:, :wc], relu,
                                        bias=b1_sb[:, c:c + 1],
                                    )
                                    hts.append(ht)

                                py = pypool.tile([P, SUBW], f32, tag="py")
                                for jj in range(wc // P):
                                    for c in range(H // P):
                                        nc.tensor.matmul(
                                            py[:, jj * P:(jj + 1) * P],
                                            lhsT=hts[c][:, jj * P:(jj + 1) * P],
                                            rhs=w2_sb[:, c, :],
                                            start=(c == 0),
                                            stop=(c == H // P - 1),
                                        )
                                yo = yopool.tile([P, SUBW // P, O], f32,
                                                 tag="yo")
                                nc.vector.tensor_tensor(
                                    out=yo[:, :wc // P, :],
                                    in0=py[:, :wc],
                                    in1=b2_sb[:, :wc],
                                    op=add,
                                )
                                nc.sync.dma_start(
                                    y[mc, ebase + o_:ebase + o_ + wc, :]
                                    .rearrange("(j p) f -> p j f", p=P),
                                    yo[:, :wc // P, :],
                                )
                        ebase += ni

    nc.compile()
    return nc


def _get_program(repeats=1):
    if repeats not in _CACHE:
        _CACHE[repeats] = _build(repeats)
    return _CACHE[repeats]


def _wrap_idx(flat):
    """[n*16k] int -> [128, n/16] int16, wrapped by 16, replicated 8x."""
    w = flat.reshape(-1, 16).T.astype(np.int16)      # [16, n/16]
    return np.tile(w, (8, 1))                        # [128, n/16]


def _prep_core(src, dst):
    """Group one core's edges by (src window, dst window).

    Returns (idx_array [128, idx_cols] int16, perm) where perm[i] = original
    edge position of padded-stream slot i (-1 for padding).
    """
    ws = src // WN
    wd = dst // WN
    g = ws * WC + wd
    order = np.argsort(g, kind="stable")
    gs = g[order]
    counts = np.bincount(gs, minlength=NG)
    if counts.max() > QUOTA:
        raise ValueError(f"group overflow: {counts.max()} > {QUOTA}")

    perm = np.full(EC_DEV, -1, dtype=np.int64)
    src_p = np.zeros(EC_DEV, dtype=np.int64)
    dst_p = np.zeros(EC_DEV, dtype=np.int64)
    pos = 0
    cols = []
    for gi in range(NG):
        n = counts[gi]
        sel = order[pos:pos + n]
        pos += n
        base = gi * QUOTA
        perm[base:base + n] = sel
        w_s, w_d = gi // WC, gi % WC
        src_p[base:base + n] = src[sel] - w_s * WN
        dst_p[base:base + n] = dst[sel] - w_d * WN
        cols.append(_wrap_idx(src_p[base:base + QUOTA]))
        cols.append(_wrap_idx(dst_p[base:base + QUOTA]))
    idx_arr = np.ascontiguousarray(np.concatenate(cols, axis=1))
    return idx_arr, perm


def kernel(edge_index, mc_embeddings, W1, b1, W2, b2):
    nc = _get_program(1)

    edge_index = np.asarray(edge_index)
    mc_embeddings = np.asarray(mc_embeddings, dtype=np.float32)
    W1 = np.asarray(W1, dtype=np.float32)
    b1 = np.asarray(b1, dtype=np.float32)
    W2 = np.asarray(W2, dtype=np.float32)
    b2 = np.asarray(b2, dtype=np.float32)

    # mc-interleaved fp16 node table: row n = [x0[n] | x1[n]].
    tab = np.ascontiguousarray(
        mc_embeddings.transpose(1, 0, 2).reshape(N_NODES, MC * D)
    ).astype(np.float16)
    w1_in = np.ascontiguousarray(W1.reshape(2, D, H)).astype(np.float16)
    w2_in = np.ascontiguousarray(W2.reshape(H // P, P, O)).astype(np.float16)
    b1_in = np.ascontiguousarray(b1.reshape(H // P, P)).astype(np.float32)
    b2_in = np.ascontiguousarray(
        np.broadcast_to(np.tile(b2, SUBW // O), (P, SUBW))
    ).astype(np.float32)

    idx64 = edge_index.astype(np.int64)
    in_maps = []
    perms = []
    for c in range(CORES):
        lo = c * E_CORE
        idx_arr, perm = _prep_core(
            idx64[0, lo:lo + E_CORE], idx64[1, lo:lo + E_CORE]
        )
        perms.append(perm)
        in_maps.append({
            "tab": tab,
            "idx": idx_arr,
            "w1": w1_in,
            "w2": w2_in,
            "b1": b1_in,
            "b2": b2_in,
        })

    global _last_in_maps
    _last_in_maps = in_maps
    res = run_bass_kernel_spmd(nc, in_maps, list(range(CORES)))

    out = np.empty((MC, E_TOTAL, O), dtype=np.float32)
    for c in range(CORES):
        lo = c * E_CORE
        yv = res.results[c]["y"]                     # [MC, EC_DEV, O]
        perm = perms[c]
        valid = perm >= 0
        out[:, lo + perm[valid], :] = yv[:, valid, :]
    return out


# revision 25
# speedup vs baseline: 1.3076x; 1.3076x over previous
"""GNN message-passing MLP on 8 Trainium2 NeuronCores.

Computes, for each of 2 "mc" embedding tables x (shape [N, 128]) and each of
500K edges (src, dst):
    y = relu(x[src] @ W1a + x[dst] @ W1b + b1) @ W2 + b2        # [2, E, 128]

Distribution: edge-parallel across 8 cores; node table + weights replicated
per core (no collectives).

Per-core kernel:
- The two mc tables are interleaved into one fp16 table [N, 256] (one 512B
  row per node serves both mc) and gathered with the GPSIMD dma_gather
  custom instruction in transpose mode, which lands gathered rows
  feature-major in SBUF ([128 feat, mc, edge]) - exactly the matmul layout.
- dma_gather indices are int16, so the table is addressed in 4 windows of
  25000 rows. The host groups each core's edges by (src window, dst window)
  into 16 groups (padded to a fixed quota) so each gather instruction hits a
  single window; outputs are produced in grouped order and inverse-permuted
  on the host.
- Layer 1 runs weight-stationary (out = W1c.T @ xT -> hT in PSUM), bias+relu
  is fused on the scalar engine (bias is per-partition in hT layout), layer 2
  contracts hT blocks against W2 chunks, and b2 is added during the
  PSUM->SBUF copy on the vector engine.
"""

import os
import sys

import numpy as np

for _p in ("/opt/trn_rl_repo", "/root/.axon_site/_ro/trn_rl_repo"):
    if os.path.isdir(_p) and _p not in sys.path:
        sys.path.insert(0, _p)

import concourse.bass as bass
import concourse.mybir as mybir
import concourse.tile as tile
from concourse import bacc
from concourse.bass_utils import run_bass_kernel_spmd
from concourse.masks import make_identity  # noqa: F401  (kept for variants)

# Problem constants (hardcoded per harness contract).
N_NODES = 100000
E_TOTAL = 500000
D = 128          # input feature dim
H = 256          # hidden dim
O = 128          # output dim
MC = 2           # number of embedding tables
CORES = 8
P = 128

# Windowed gather layout.
WN = 25000                   # nodes per index window (int16-addressable)
WC = 4                       # windows
NG = WC * WC                 # (src window, dst window) groups
QUOTA = 4352                 # padded edges per group (mean 3906, +7.4 sigma)
EC_DEV = NG * QUOTA          # padded edges per core (69632)
E_CORE = E_TOTAL // CORES    # real edges per core (62500)
CHUNKS = (2048, 2048, 256)   # gather instruction sizes per group (sum=QUOTA)
SUBW = 512                   # compute batch width (edges per L1 matmul)

_CACHE = {}
_last_in_maps = None


def _build(repeats=1):
    f16 = mybir.dt.float16
    f32 = mybir.dt.float32
    i16 = mybir.dt.int16

    idx_cols_per_group = 2 * (QUOTA // 16)       # src + dst, wrapped by 16
    idx_cols = NG * idx_cols_per_group           # 8704

    # NOTE: num_swdge_queues=2 with gathers split across queues measures
    # ~1.32 ms/pass (vs 1.70 ms) but produces wrong results on hardware
    # (queue-1 gather completions are not correctly awaited) - keep 1 queue.
    nc = bacc.Bacc("TRN2", target_bir_lowering=False, num_devices=CORES)
    tab = nc.declare_dram_parameter("tab", [N_NODES, MC * D], f16, isOutput=False)
    idx = nc.declare_dram_parameter("idx", [P, idx_cols], i16, isOutput=False)
    w1 = nc.declare_dram_parameter("w1", [2, D, H], f16, isOutput=False)
    w2 = nc.declare_dram_parameter("w2", [H // P, P, O], f16, isOutput=False)
    b1 = nc.declare_dram_parameter("b1", [H // P, P], f32, isOutput=False)
    b2 = nc.declare_dram_parameter("b2", [P, SUBW], f32, isOutput=False)
    y = nc.declare_dram_parameter("y", [MC, EC_DEV, O], f32, isOutput=True)

    relu = mybir.ActivationFunctionType.Relu
    add = mybir.AluOpType.add

    with tile.TileContext(nc) as tc:
        with (
            tc.tile_pool(name="const", bufs=1) as cpool,
            tc.tile_pool(name="xg", bufs=2) as xgpool,
            tc.tile_pool(name="ht", bufs=3) as htpool,
            tc.tile_pool(name="yo", bufs=3) as yopool,
            tc.tile_pool(name="ph", bufs=2, space="PSUM") as phpool,
            tc.tile_pool(name="py", bufs=2, space="PSUM") as pypool,
        ):
            w1_sb = cpool.tile([P, 2, H], f16)       # [d, a/b, h]
            nc.sync.dma_start(w1_sb[:], w1.rearrange("a d h -> d a h"))
            w2_sb = cpool.tile([P, H // P, O], f16)  # [h_in_chunk, chunk, o]
            nc.sync.dma_start(w2_sb[:], w2.rearrange("c h o -> h c o"))
            b1_sb = cpool.tile([P, H // P], f32)
            nc.sync.dma_start(b1_sb[:], b1.rearrange("c p -> p c"))
            b2_sb = cpool.tile([P, SUBW], f32)
            nc.sync.dma_start(b2_sb[:], b2[:])
            ix_all = cpool.tile([P, idx_cols], i16)
            nc.sync.dma_start(ix_all[:], idx[:])

            for _rep in range(repeats):
                for g in range(NG):
                    ws, wd = g // WC, g % WC
                    gbase = g * QUOTA                    # edge-stream offset
                    icol = g * idx_cols_per_group        # idx-column offset
                    ebase = gbase
                    for ni in CHUNKS:
                        s_cols = ni // 16
                        # Gather src rows (transpose mode -> feature-major).
                        xst = xgpool.tile([P, MC, ni], f16, tag="xst")
                        nc.gpsimd.dma_gather(
                            out_ap=xst[:],
                            in_ap=tab[ws * WN:(ws + 1) * WN, :],
                            idxs_ap=ix_all[:, icol:icol + s_cols],
                            num_idxs=ni,
                            num_idxs_reg=ni,
                            elem_size=MC * D,
                            transpose=True,
                            single_packet=False,
                        )
                        xdt = xgpool.tile([P, MC, ni], f16, tag="xdt")
                        nc.gpsimd.dma_gather(
                            out_ap=xdt[:],
                            in_ap=tab[wd * WN:(wd + 1) * WN, :],
                            idxs_ap=ix_all[:, icol + QUOTA // 16:
                                           icol + QUOTA // 16 + s_cols],
                            num_idxs=ni,
                            num_idxs_reg=ni,
                            elem_size=MC * D,
                            transpose=True,
                            single_packet=False,
                        )
                        icol += s_cols

                        for o_ in range(0, ni, SUBW):
                            wc = min(SUBW, ni - o_)
                            for mc in range(MC):
                                hts = []
                                for c in range(H // P):
                                    ph = phpool.tile([P, SUBW], f32,
                                                     tag=f"ph{c}")
                                    nc.tensor.matmul(
                                        ph[:, :wc],
                                        lhsT=w1_sb[:, 0, c * P:(c + 1) * P],
                                        rhs=xst[:, mc, o_:o_ + wc],
                                        start=True, stop=False,
                                    )
                                    nc.tensor.matmul(
                                        ph[:, :wc],
                                        lhsT=w1_sb[:, 1, c * P:(c + 1) * P],
                                        rhs=xdt[:, mc, o_:o_ + wc],
                                        start=False, stop=True,
                                    )
                                    ht = htpool.tile([P, SUBW], f16,
                                                     tag=f"ht{c}")
                                    nc.scalar.activation(
                                        ht[:, :wc], ph[:, :wc], relu,
                                        bias=b1_sb[:, c:c + 1],
                                    )
                                    hts.append(ht)

                                py = pypool.tile([P, SUBW], f32, tag="py")
                                for jj in range(wc // P):
                                    for c in range(H // P):
                                        nc.tensor.matmul(
                                            py[:, jj * P:(jj + 1) * P],
                                            lhsT=hts[c][:, jj * P:(jj + 1) * P],
                                            rhs=w2_sb[:, c, :],
                                            start=(c == 0),
                                            stop=(c == H // P - 1),
                                        )
                                yo = yopool.tile([P, SUBW // P, O], f32,
                                                 tag="yo")
                                nc.vector.tensor_tensor(
                                    out=yo[:, :wc // P, :],
                                    in0=py[:, :wc],
                                    in1=b2_sb[:, :wc],
                                    op=add,
                                )
                                nc.sync.dma_start(
                                    y[mc, ebase + o_:ebase + o_ + wc, :]
                                    .rearrange("(j p) f -> p j f", p=P),
                                    yo[:, :wc // P, :],
                                )
                        ebase += ni

    nc.compile()
    return nc


def _get_program(repeats=1):
    if repeats not in _CACHE:
        _CACHE[repeats] = _build(repeats)
    return _CACHE[repeats]


def _wrap_idx(flat):
    """[n*16k] int -> [128, n/16] int16, wrapped by 16, replicated 8x."""
    w = flat.reshape(-1, 16).T.astype(np.int16)      # [16, n/16]
    return np.tile(w, (8, 1))                        # [128, n/16]


def _prep_core(src, dst):
    """Group one core's edges by (src window, dst window).

    Returns (idx_array [128, idx_cols] int16, perm) where perm[i] = original
    edge position of padded-stream slot i (-1 for padding).
    """
    ws = src // WN
    wd = dst // WN
    g = ws * WC + wd
    order = np.argsort(g, kind="stable")
    gs = g[order]
    counts = np.bincount(gs, minlength=NG)
    if counts.max() > QUOTA:
        raise ValueError(f"group overflow: {counts.max()} > {QUOTA}")

    perm = np.full(EC_DEV, -1, dtype=np.int64)
    src_p = np.zeros(EC_DEV, dtype=np.int64)
    dst_p = np.zeros(EC_DEV, dtype=np.int64)
    pos = 0
    cols = []
    for gi in range(NG):
        n = counts[gi]
        sel = order[pos:pos + n]
        pos += n
        base = gi * QUOTA
        perm[base:base + n] = sel
        w_s, w_d = gi // WC, gi % WC
        src_p[base:base + n] = src[sel] - w_s * WN
        dst_p[base:base + n] = dst[sel] - w_d * WN
        cols.append(_wrap_idx(src_p[base:base + QUOTA]))
        cols.append(_wrap_idx(dst_p[base:base + QUOTA]))
    idx_arr = np.ascontiguousarray(np.concatenate(cols, axis=1))
    return idx_arr, perm


def kernel(edge_index, mc_embeddings, W1, b1, W2, b2):
    nc = _get_program(1)

    edge_index = np.asarray(edge_index)
    mc_embeddings = np.asarray(mc_embeddings, dtype=np.float32)
    W1 = np.asarray(W1, dtype=np.float32)
    b1 = np.asarray(b1, dtype=np.float32)
    W2 = np.asarray(W2, dtype=np.float32)
    b2 = np.asarray(b2, dtype=np.float32)

    # mc-interleaved fp16 node table: row n = [x0[n] | x1[n]].
    tab = np.ascontiguousarray(
        mc_embeddings.transpose(1, 0, 2).reshape(N_NODES, MC * D)
    ).astype(np.float16)
    w1_in = np.ascontiguousarray(W1.reshape(2, D, H)).astype(np.float16)
    w2_in = np.ascontiguousarray(W2.reshape(H // P, P, O)).astype(np.float16)
    b1_in = np.ascontiguousarray(b1.reshape(H // P, P)).astype(np.float32)
    b2_in = np.ascontiguousarray(
        np.broadcast_to(np.tile(b2, SUBW // O), (P, SUBW))
    ).astype(np.float32)

    idx64 = edge_index.astype(np.int64)
    in_maps = []
    perms = []
    for c in range(CORES):
        lo = c * E_CORE
        idx_arr, perm = _prep_core(
            idx64[0, lo:lo + E_CORE], idx64[1, lo:lo + E_CORE]
        )
        perms.append(perm)
        in_maps.append({
            "tab": tab,
            "idx": idx_arr,
            "w1": w1_in,
            "w2": w2_in,
            "b1": b1_in,
            "b2": b2_in,
        })

    global _last_in_maps
    _last_in_maps = in_maps
    res = run_bass_kernel_spmd(nc, in_maps, list(range(CORES)))

    out = np.empty((MC, E_TOTAL, O), dtype=np.float32)
    for c in range(CORES):
        lo = c * E_CORE
        yv = res.results[c]["y"]                     # [MC, EC_DEV, O]
        perm = perms[c]
        valid = perm >= 0
        out[:, lo + perm[valid], :] = yv[:, valid, :]
    return out
